# revision 38
# baseline (speedup 1.0000x reference)
"""Trainium2 Bass kernel for a 2-layer GCN (PyG GCNConv x2 with self-loops).

Reference computation (N=100000 nodes, E=1600000 edges, f32):
    row, col = add_self_loops(edge_index)
    deg  = in-degree over col (incl. self loops); dis = rsqrt(deg)
    norm = dis[row] * dis[col]
    A_hat X = segment_sum(X[row] * norm, col)          # normalized aggregation
    h   = relu(A_hat X @ W1 + b1)                      # aggregate-then-transform
    out = (A_hat h) @ W2 + b2

Key algebraic identity used: segment_sum((X W)[row]*norm, col) ==
segment_sum(X[row]*norm, col) @ W, so aggregation happens in the *input*
feature dim (64 / 128) which halves gather traffic.

Distribution over 8 NeuronCores: destination-node sharding (12500 nodes
per core).  Each core aggregates messages for its own destination shard,
computes its shard of h, then 4 chunked AllGathers replicate h (fp16) to
every core for the layer-2 gather.

Per-core aggregation strategy (no scatter / no races):
  - edges (incl. self-loops) are bucketed host-side by (source-chunk k,
    dest-window w); source chunks keep dma_gather's int16 indices in
    range; dest windows are 128 destinations wide.
  - device gathers source rows with gpsimd.dma_gather (elem=256B),
    builds a per-128-edge-chunk selection matrix P[e, d] =
    norm_e * (dest_e == d) with two vector ops (iota compare + scale),
    and accumulates PSUM[feat, dest] += msg_chunk.T @ P on the tensor
    engine.  The PSUM result is already transposed ([feat, dest]) which
    is exactly the lhsT layout the following dense matmul needs.

All cores run one identical program (SPMD); per-(k,w) chunk counts are
equalized across cores host-side with zero-norm padding edges.
"""

import os
import sys

import numpy as np

for _p in ("/opt/trn_rl_repo", "/root/.axon_site/_ro/trn_rl_repo"):
    if os.path.isdir(_p) and _p not in sys.path:
        sys.path.insert(0, _p)

# ----------------------------------------------------------------------------
# Problem constants (hardcoded per the harness contract)
# ----------------------------------------------------------------------------
N = 100000          # nodes
NC = 8              # cores
NS = N // NC        # 12500 dest nodes per core
D0, D1, D2 = 64, 128, 256
WIN = 128           # dest window width
NW = (NS + WIN - 1) // WIN          # 98 windows per core
NSRC = 4            # source chunks (int16 gather index limit)
SC1 = N // NSRC     # 25000 rows per layer-1 source chunk
Q4 = NS // NSRC     # 3125 rows per all-gather chunk slice
G = 1024            # edges per dma_gather call; hard cap: the SWDGE ring
                    # holds dynamic_dma_scratch_size/16 = 1024 descriptors
                    # and a larger call deadlocks on hardware
NQ = 4              # SWDGE queues (ucode max); gather calls round-robin
_SINGLE_PACKET = os.environ.get("GCN_SINGLE_PACKET", "1") == "1"


# ----------------------------------------------------------------------------
# Host-side preprocessing: sharding + edge bucketing
# ----------------------------------------------------------------------------
def _plan_layer(src_all, dloc_all, norm_all, core_all, src_chunk_of, src_idx_of,
                meta_np_dtype=np.float32):
    """Bucket each core's edges by (source-chunk k, dest-window w), pad each
    bucket to a multiple of 128 edges AND to identical chunk counts across
    all cores (SPMD requires one program).

    Returns (meta, per_core_arrays):
      meta = {"Tk": [T_0..T_3], "segs": [[(w, n_chunks), ...] per k]}
      per_core_arrays[c] = {"idx": [4 arrays int16 [16, T/16]],
                            "dst": [4 arrays f32 [128, T/128]],
                            "nrm": [4 arrays f32 [128, T/128]]}
    """
    counts = np.zeros((NC, NSRC, NW), dtype=np.int64)
    per_core = []
    for c in range(NC):
        sel = core_all == c
        src = src_all[sel]
        dloc = dloc_all[sel]
        nrm = norm_all[sel]
        k = src_chunk_of(src)
        w = dloc // WIN
        idxl = src_idx_of(src)
        order = np.lexsort((w, k))
        k, w, idxl, dloc, nrm = k[order], w[order], idxl[order], dloc[order], nrm[order]
        key = k * NW + w
        counts[c] = np.bincount(key, minlength=NSRC * NW).reshape(NSRC, NW)
        per_core.append((k, w, idxl, dloc, nrm, key))

    nch = (counts.max(axis=0) + 127) // 128          # [NSRC, NW] chunks per bucket
    Tk = (nch.sum(axis=1) * 128).astype(np.int64)    # padded edges per chunk k
    segs = [[(int(w), int(nch[k, w])) for w in range(NW) if nch[k, w] > 0]
            for k in range(NSRC)]

    # bucket base offsets inside each k-stream (in edges)
    base = np.zeros((NSRC, NW), dtype=np.int64)
    for k in range(NSRC):
        base[k] = np.concatenate(([0], np.cumsum(nch[k] * 128)[:-1]))

    out = []
    for c in range(NC):
        k, w, idxl, dloc, nrm, key = per_core[c]
        cnt = counts[c].reshape(-1)
        starts = np.concatenate(([0], np.cumsum(cnt)[:-1]))
        pos_in_bucket = np.arange(len(key)) - starts[key]
        tgt = base.reshape(-1)[key] + pos_in_bucket   # position inside k-stream
        arrs = {"idx": [], "dst": [], "nrm": []}
        for kk in range(NSRC):
            T = int(Tk[kk])
            idx16 = np.zeros(T, dtype=np.int16)
            dwf = np.full(T, -1.0, dtype=np.float32)
            nwf = np.zeros(T, dtype=np.float32)
            m = k == kk
            t = tgt[m]
            idx16[t] = idxl[m].astype(np.int16)
            dwf[t] = (dloc[m] - (w[m] * WIN)).astype(np.float32)
            nwf[t] = nrm[m]
            # device layouts: idx wraps by 16 (replicated to 128 partitions
            # for the 8 gpsimd cores), dst/nrm wrap by 128
            arrs["idx"].append(np.ascontiguousarray(
                np.tile(idx16.reshape(T // 16, 16).T, (8, 1))))
            arrs["dst"].append(np.ascontiguousarray(
                dwf.reshape(T // 128, 128).T.astype(meta_np_dtype)))
            arrs["nrm"].append(np.ascontiguousarray(
                nwf.reshape(T // 128, 128).T.astype(meta_np_dtype)))
        out.append(arrs)
    return {"Tk": [int(t) for t in Tk], "segs": segs}, out


def _preprocess(x, edge_index, W1, b1, W2, b2):
    row = np.asarray(edge_index[0], dtype=np.int64)
    col = np.asarray(edge_index[1], dtype=np.int64)
    deg = (np.bincount(col, minlength=N) + 1).astype(np.float32)  # + self loop
    dis = (1.0 / np.sqrt(deg)).astype(np.float32)

    # self-loops are handled by a separate descriptor-free streamed bucket
    rows, cols = row, col
    norms = (dis[rows] * dis[cols]).astype(np.float32)
    core = (cols // NS).astype(np.int64)
    dloc = cols - core * NS

    meta1, arrs1 = _plan_layer(
        rows, dloc, norms, core,
        src_chunk_of=lambda s: s // SC1,
        src_idx_of=lambda s: s % SC1,
    )
    meta2, arrs2 = _plan_layer(
        rows, dloc, norms, core,
        src_chunk_of=lambda s: (s % NS) // Q4,
        src_idx_of=lambda s: (s // NS) * Q4 + (s % NS) % Q4,
        meta_np_dtype=np.float16,
    )

    x = np.ascontiguousarray(np.asarray(x, dtype=np.float32))
    shared = {
        "x": x,
        "W1": np.ascontiguousarray(np.asarray(W1, dtype=np.float32)),
        "b1": np.ascontiguousarray(np.asarray(b1, dtype=np.float32).reshape(1, D1)),
        "W2": np.ascontiguousarray(np.asarray(W2, dtype=np.float32)),
        "b2": np.ascontiguousarray(np.asarray(b2, dtype=np.float32).reshape(1, D2)),
    }
    # self-loop bucket: identical structure on every core. dst within window
    # is simply the partition index; norm is dis^2 of the node.
    NPAD = NW * WIN
    p_i = np.arange(WIN)[:, None]
    w_i = np.arange(NW)[None, :]
    node = w_i * WIN + p_i                       # [WIN, NW] local node id
    sdst = np.where(node < NS, p_i, -1.0)
    in_maps = []
    for c in range(NC):
        m = dict(shared)
        dis2 = np.zeros((WIN, NW), np.float32)
        valid = node < NS
        gnode = c * NS + np.minimum(node, NS - 1)
        dis2[valid] = (dis[gnode] ** 2)[valid]
        m["sdst1"] = np.ascontiguousarray(sdst.astype(np.float16))
        m["snrm1"] = np.ascontiguousarray(dis2.astype(np.float16))
        m["sdst2"] = m["sdst1"]
        m["snrm2"] = m["snrm1"]
        xo = np.zeros((NPAD, D0), np.float16)
        xo[:NS] = x[c * NS:(c + 1) * NS]
        # shuffled [128, NW*D0] so the device load is one contiguous DMA
        m["xown"] = np.ascontiguousarray(
            xo.reshape(NW, WIN, D0).transpose(1, 0, 2).reshape(WIN, NW * D0))
        for kk in range(NSRC):
            m[f"idx1_{kk}"] = arrs1[c]["idx"][kk]
            m[f"dst1_{kk}"] = arrs1[c]["dst"][kk]
            m[f"nrm1_{kk}"] = arrs1[c]["nrm"][kk]
            m[f"idx2_{kk}"] = arrs2[c]["idx"][kk]
            m[f"dst2_{kk}"] = arrs2[c]["dst"][kk]
            m[f"nrm2_{kk}"] = arrs2[c]["nrm"][kk]
        in_maps.append(m)
    return meta1, meta2, in_maps


# ----------------------------------------------------------------------------
# Device program
# ----------------------------------------------------------------------------
def _aggregate(nc, tc, mybir, meta, idx_d, dst_d, nrm_d, src_aps, acc, feat,
               dt_g, iota_t, tag, self_src=None, sdst_d=None, snrm_d=None,
               iota16=None):
    """Gather + P-matmul aggregation of one layer into SBUF acc [feat, NW*WIN].

    The P (one-hot * norm) selection tensor is built for a whole gather
    tile (GC chunks = G edges) with TWO vector ops, using stride-0
    broadcast access patterns on the per-edge dest/norm columns.
    """
    from contextlib import ExitStack

    import concourse.bass as bass

    f32 = mybir.dt.float32
    GC = G // 128
    with ExitStack() as ctx:
        mp = ctx.enter_context(tc.tile_pool(name=f"meta{tag}", bufs=2))
        gp = ctx.enter_context(tc.tile_pool(name=f"g{tag}", bufs=8))
        pp = ctx.enter_context(tc.tile_pool(name=f"p{tag}", bufs=4))
        psp = ctx.enter_context(tc.tile_pool(name=f"ps{tag}", bufs=4, space="PSUM"))

        def bcast(col_slice, mc):
            return bass.AP(col_slice.tensor, col_slice.offset,
                           [list(col_slice.ap[0]), [1, mc], [0, WIN]])

        # descriptor-free self-loop bucket: per-core own rows streamed with
        # one plain DMA; P is the diag(dis^2) selection per window (fp16).
        if self_src is not None:
            f16 = mybir.dt.float16
            sdst_t = mp.tile([128, NW], f16, tag=f"sdst{tag}")
            nc.sync.dma_start(sdst_t[:], sdst_d[:])
            snrm_t = mp.tile([128, NW], f16, tag=f"snrm{tag}")
            nc.sync.dma_start(snrm_t[:], snrm_d[:])
            stp = ctx.enter_context(tc.tile_pool(name=f"st{tag}", bufs=1))
            st = stp.tile([128, NW * feat], f16)
            nc.sync.dma_start(st[:], self_src[:])
            GC8 = G // 128
            for wg in range(0, NW, GC8):
                mc = min(GC8, NW - wg)
                P8s = pp.tile([128, GC8, WIN], f16, tag=f"Ps{tag}")
                nc.vector.tensor_tensor(
                    P8s[:, :mc, :], iota16[:, :mc, :],
                    bcast(sdst_t[:, wg:wg + mc], mc),
                    mybir.AluOpType.is_equal)
                nc.vector.tensor_tensor(
                    P8s[:, :mc, :], P8s[:, :mc, :],
                    bcast(snrm_t[:, wg:wg + mc], mc),
                    mybir.AluOpType.mult)
                for i in range(mc):
                    w = wg + i
                    ps = psp.tile([feat, WIN], mybir.dt.float32)
                    nc.tensor.matmul(ps[:], st[:, w * feat:(w + 1) * feat],
                                     P8s[:, i, :], start=True, stop=True)
                    nc.vector.tensor_tensor(
                        acc[:, w * WIN:(w + 1) * WIN],
                        acc[:, w * WIN:(w + 1) * WIN], ps[:],
                        mybir.AluOpType.add)

        ncalls = 0
        for k in range(NSRC):
            Tk = meta["Tk"][k]
            if Tk == 0:
                continue
            idx_t = mp.tile([128, Tk // 16], mybir.dt.int16, tag=f"idx{tag}")
            nc.sync.dma_start(idx_t[:], idx_d[k][:])
            dst_t = mp.tile([128, Tk // 128], dt_g, tag=f"dst{tag}")
            nc.sync.dma_start(dst_t[:], dst_d[k][:])
            nrm_t = mp.tile([128, Tk // 128], dt_g, tag=f"nrm{tag}")
            nc.sync.dma_start(nrm_t[:], nrm_d[k][:])
            jj = 0
            gt = None
            P8 = None
            for (w, nchk) in meta["segs"][k]:
                ps = psp.tile([feat, WIN], f32)
                for j in range(nchk):
                    g, slot = divmod(jj, GC)
                    if slot == 0:
                        mlen = min(G, Tk - g * G)
                        mc = mlen // 128
                        gt = gp.tile([128, GC, feat], dt_g, tag=f"gt{tag}")
                        nc.gpsimd.dma_gather(
                            gt[:, :mc, :],
                            src_aps[k],
                            idx_t[:, g * (G // 16): (g * G + mlen) // 16],
                            mlen, mlen, feat, elem_step=feat,
                            queue_num=ncalls % NQ,
                            single_packet=_SINGLE_PACKET,
                        )
                        ncalls += 1
                        P8 = pp.tile([128, GC, WIN], dt_g, tag=f"P{tag}")
                        nc.vector.tensor_tensor(
                            P8[:, :mc, :], iota_t[:, :mc, :],
                            bcast(dst_t[:, jj:jj + mc], mc),
                            mybir.AluOpType.is_equal)
                        nc.vector.tensor_tensor(
                            P8[:, :mc, :], P8[:, :mc, :],
                            bcast(nrm_t[:, jj:jj + mc], mc),
                            mybir.AluOpType.mult)
                    nc.tensor.matmul(ps[:], gt[:, slot, :], P8[:, slot, :],
                                     start=(j == 0), stop=(j == nchk - 1))
                    jj += 1
                nc.vector.tensor_tensor(
                    acc[:, w * WIN:(w + 1) * WIN],
                    acc[:, w * WIN:(w + 1) * WIN], ps[:],
                    mybir.AluOpType.add)


def _build(meta1, meta2, debug=False, stage="full"):
    from contextlib import ExitStack

    import concourse.bacc as bacc
    import concourse.mybir as mybir
    import concourse.tile as tile

    f32, f16, i16 = mybir.dt.float32, mybir.dt.float16, mybir.dt.int16

    nc = bacc.Bacc("TRN2", target_bir_lowering=False, debug=debug,
                   num_devices=NC, num_swdge_queues=NQ)

    x_d = nc.dram_tensor("x", [N, D0], f32, kind="ExternalInput")
    w1_d = nc.dram_tensor("W1", [D0, D1], f32, kind="ExternalInput")
    b1_d = nc.dram_tensor("b1", [1, D1], f32, kind="ExternalInput")
    w2_d = nc.dram_tensor("W2", [D1, D2], f32, kind="ExternalInput")
    b2_d = nc.dram_tensor("b2", [1, D2], f32, kind="ExternalInput")

    idx1_d, dst1_d, nrm1_d, idx2_d, dst2_d, nrm2_d = [], [], [], [], [], []
    for k in range(NSRC):
        T1, T2 = meta1["Tk"][k], meta2["Tk"][k]
        idx1_d.append(nc.dram_tensor(f"idx1_{k}", [128, T1 // 16], i16, kind="ExternalInput"))
        dst1_d.append(nc.dram_tensor(f"dst1_{k}", [128, T1 // 128], f32, kind="ExternalInput"))
        nrm1_d.append(nc.dram_tensor(f"nrm1_{k}", [128, T1 // 128], f32, kind="ExternalInput"))
        idx2_d.append(nc.dram_tensor(f"idx2_{k}", [128, T2 // 16], i16, kind="ExternalInput"))
        dst2_d.append(nc.dram_tensor(f"dst2_{k}", [128, T2 // 128], f16, kind="ExternalInput"))
        nrm2_d.append(nc.dram_tensor(f"nrm2_{k}", [128, T2 // 128], f16, kind="ExternalInput"))

    xown_d = nc.dram_tensor("xown", [128, NW * D0], f16, kind="ExternalInput")
    h_shuf = nc.dram_tensor("h_shuf", [128, NW * D1], f16, kind="Internal")
    sdst1_d = nc.dram_tensor("sdst1", [128, NW], f16, kind="ExternalInput")
    snrm1_d = nc.dram_tensor("snrm1", [128, NW], f16, kind="ExternalInput")
    sdst2_d = nc.dram_tensor("sdst2", [128, NW], f16, kind="ExternalInput")
    snrm2_d = nc.dram_tensor("snrm2", [128, NW], f16, kind="ExternalInput")
    h_own = nc.dram_tensor("h_own", [NW * WIN, D1], f16, kind="Internal")
    hf = [nc.dram_tensor(f"hf{q}", [NC * Q4, D1], f16, kind="Internal",
                         addr_space="Shared") for q in range(NSRC)]
    if stage == "A":
        out_d = nc.dram_tensor("out", [D0, NW * WIN], f32, kind="ExternalOutput")
    elif stage == "AB":
        out_d = nc.dram_tensor("out", [NS, D1], f16, kind="ExternalOutput")
    elif stage == "ABC":
        out_d = nc.dram_tensor("out", [NC * Q4, D1], f16, kind="ExternalOutput")
    elif stage == "ABCD":
        out_d = nc.dram_tensor("out", [D1, NW * WIN], f32, kind="ExternalOutput")
    else:
        out_d = nc.dram_tensor("out", [NS, D2], f32, kind="ExternalOutput")

    with tile.TileContext(nc) as tc:
        with ExitStack() as top:
            const = top.enter_context(tc.tile_pool(name="const", bufs=1))
            w1_t = const.tile([D0, D1], f32)
            nc.sync.dma_start(w1_t[:], w1_d[:])
            b1_t = const.tile([1, D1], f32)
            nc.sync.dma_start(b1_t[:], b1_d[:])
            w2_t = const.tile([D1, D2], f32)
            nc.sync.dma_start(w2_t[:], w2_d[:])
            b2_t = const.tile([1, D2], f32)
            nc.sync.dma_start(b2_t[:], b2_d[:])
            GC = G // 128
            iota32 = const.tile([128, GC, WIN], f32)
            nc.gpsimd.iota(iota32[:], pattern=[[0, GC], [1, WIN]], base=0,
                           channel_multiplier=0,
                           allow_small_or_imprecise_dtypes=True)
            iota16 = const.tile([128, GC, WIN], f16)
            nc.gpsimd.iota(iota16[:], pattern=[[0, GC], [1, WIN]], base=0,
                           channel_multiplier=0,
                           allow_small_or_imprecise_dtypes=True)
            ones_t = const.tile([1, 128], f32)
            nc.vector.memset(ones_t[:], 1.0)

            accp = top.enter_context(tc.tile_pool(name="acc", bufs=1))
            acc2 = accp.tile([D1, NW * WIN], f32)

            # ---- Layer 1: aggregate into acc1 [D0, NW*WIN] ----
            with tc.tile_pool(name="acc1", bufs=1) as acc1p:
                acc1 = acc1p.tile([D0, NW * WIN], f32)
                nc.vector.memset(acc1[:], 0.0)
                _aggregate(nc, tc, mybir, meta1, idx1_d, dst1_d, nrm1_d,
                           [x_d[k * SC1:(k + 1) * SC1, :] for k in range(NSRC)],
                           acc1, D0, f32, iota32, "A",
                           self_src=xown_d, sdst_d=sdst1_d, snrm_d=snrm1_d,
                           iota16=iota16)

                if stage == "A":
                    nc.sync.dma_start(out_d[:], acc1[:])
                else:
                    # ---- h = relu(acc1.T @ W1 + b1), store fp16 ----
                    with ExitStack() as sb:
                        hp = sb.enter_context(tc.tile_pool(name="hb", bufs=4))
                        psb = sb.enter_context(tc.tile_pool(name="psb", bufs=4, space="PSUM"))
                        for w in range(NW):
                            ps = psb.tile([WIN, D1], f32, tag="psb")
                            nc.tensor.matmul(ps[:], acc1[:, w * WIN:(w + 1) * WIN],
                                             w1_t[:], start=True, stop=False)
                            nc.tensor.matmul(ps[:], ones_t[:, :WIN], b1_t[:],
                                             start=False, stop=True)
                            ht = hp.tile([WIN, D1], f16, tag="ht")
                            nc.scalar.activation(ht[:], ps[:],
                                                 mybir.ActivationFunctionType.Relu)
                            nc.sync.dma_start(h_own[w * WIN:(w + 1) * WIN, :], ht[:])
                            nc.sync.dma_start(h_shuf[:, w * D1:(w + 1) * D1], ht[:])

            if stage == "AB":
                nc.sync.dma_start(out_d[:], h_own[:])
            elif stage != "A":
                nc.vector.memset(acc2[:], 0.0)

                # ---- AllGather h (4 node-range chunks) ----
                for q in range(NSRC):
                    nc.gpsimd.collective_compute(
                        "AllGather", mybir.AluOpType.bypass,
                        replica_groups=[list(range(NC))],
                        ins=[h_own[q * Q4:(q + 1) * Q4, :]],
                        outs=[hf[q][:, :]],
                    )

                if stage == "ABC":
                    nc.sync.dma_start(out_d[:], hf[0][:])
                else:
                    # ---- Layer 2: aggregate into acc2 [D1, NW*WIN] ----
                    _aggregate(nc, tc, mybir, meta2, idx2_d, dst2_d, nrm2_d,
                               [hf[k][:, :] for k in range(NSRC)],
                               acc2, D1, f16, iota16, "B",
                               self_src=h_shuf, sdst_d=sdst2_d, snrm_d=snrm2_d,
                               iota16=iota16)

                    if stage == "ABCD":
                        nc.sync.dma_start(out_d[:], acc2[:])
                    else:
                        # ---- out = acc2.T @ W2 + b2 ----
                        with ExitStack() as sb:
                            op = sb.enter_context(tc.tile_pool(name="ob", bufs=4))
                            pso = sb.enter_context(tc.tile_pool(name="pso", bufs=4, space="PSUM"))
                            for w in range(NW):
                                M = min(WIN, NS - w * WIN)
                                ps = pso.tile([M, D2], f32, tag="pso")
                                nc.tensor.matmul(ps[:], acc2[:, w * WIN:w * WIN + M],
                                                 w2_t[:], start=True, stop=False)
                                nc.tensor.matmul(ps[:], ones_t[:, :M], b2_t[:],
                                                 start=False, stop=True)
                                ot = op.tile([M, D2], f32, tag="ot")
                                nc.vector.tensor_copy(ot[:], ps[:])
                                nc.sync.dma_start(out_d[w * WIN:w * WIN + M, :], ot[:])

    nc.compile()
    return nc


# ----------------------------------------------------------------------------
# Entry point
# ----------------------------------------------------------------------------
def _ensure_axon_hooks_module():
    """bass_utils hard-imports antenv.axon_hooks when BASS_TRACE is set;
    provide a degradable stub if the image's antenv lacks it."""
    import types

    try:
        import antenv.axon_hooks  # noqa: F401
        return
    except ImportError:
        pass
    try:
        import antenv
    except ImportError:
        return
    mod = types.ModuleType("antenv.axon_hooks")
    mod._hook = None
    mod.set_axon_ntff_profile_hook = lambda h: setattr(mod, "_hook", h)
    mod.get_axon_ntff_profile_hook = lambda: mod._hook
    sys.modules["antenv.axon_hooks"] = mod
    antenv.axon_hooks = mod


def kernel(x, edge_index, W1, b1, W2, b2):
    _ensure_axon_hooks_module()
    from concourse import bass_utils

    meta1, meta2, in_maps = _preprocess(x, edge_index, W1, b1, W2, b2)
    nc = _build(meta1, meta2, debug=False)
    res = bass_utils.run_bass_kernel_spmd(nc, in_maps, core_ids=list(range(NC)))
    out = np.concatenate([r["out"] for r in res.results], axis=0)
    return out.astype(np.float32)


# revision 43
# speedup vs baseline: 1.2897x; 1.2897x over previous
"""Trainium2 Bass kernel for a 2-layer GCN (PyG GCNConv x2 with self-loops).

Reference computation (N=100000 nodes, E=1600000 edges, f32):
    row, col = add_self_loops(edge_index)
    deg  = in-degree over col (incl. self loops); dis = rsqrt(deg)
    norm = dis[row] * dis[col]
    A_hat X = segment_sum(X[row] * norm, col)          # normalized aggregation
    h   = relu(A_hat X @ W1 + b1)                      # aggregate-then-transform
    out = (A_hat h) @ W2 + b2

Key algebraic identity used: segment_sum((X W)[row]*norm, col) ==
segment_sum(X[row]*norm, col) @ W, so aggregation happens in the *input*
feature dim (64 / 128) which halves gather traffic.

Distribution over 8 NeuronCores: destination-node sharding (12500 nodes
per core).  Each core aggregates messages for its own destination shard,
computes its shard of h, then 4 chunked AllGathers replicate h (fp16) to
every core for the layer-2 gather.

Per-core aggregation strategy (no scatter / no races):
  - edges (incl. self-loops) are bucketed host-side by (source-chunk k,
    dest-window w); source chunks keep dma_gather's int16 indices in
    range; dest windows are 128 destinations wide.
  - device gathers source rows with gpsimd.dma_gather (elem=256B),
    builds a per-128-edge-chunk selection matrix P[e, d] =
    norm_e * (dest_e == d) with two vector ops (iota compare + scale),
    and accumulates PSUM[feat, dest] += msg_chunk.T @ P on the tensor
    engine.  The PSUM result is already transposed ([feat, dest]) which
    is exactly the lhsT layout the following dense matmul needs.

All cores run one identical program (SPMD); per-(k,w) chunk counts are
equalized across cores host-side with zero-norm padding edges.
"""

import os
import sys

import numpy as np

for _p in ("/opt/trn_rl_repo", "/root/.axon_site/_ro/trn_rl_repo"):
    if os.path.isdir(_p) and _p not in sys.path:
        sys.path.insert(0, _p)

# ----------------------------------------------------------------------------
# Problem constants (hardcoded per the harness contract)
# ----------------------------------------------------------------------------
N = 100000          # nodes
NC = 8              # cores
NS = N // NC        # 12500 dest nodes per core
D0, D1, D2 = 64, 128, 256
WIN = 128           # dest window width
NW = (NS + WIN - 1) // WIN          # 98 windows per core
NSRC = 4            # source chunks (int16 gather index limit)
SC1 = N // NSRC     # 25000 rows per layer-1 source chunk
Q4 = NS // NSRC     # 3125 rows per all-gather chunk slice
G = 1024            # edges per dma_gather call; hard cap: the SWDGE ring
                    # holds dynamic_dma_scratch_size/16 = 1024 descriptors
                    # and a larger call deadlocks on hardware
NQ = 4              # SWDGE queues (ucode max); gather calls round-robin
_SINGLE_PACKET = os.environ.get("GCN_SINGLE_PACKET", "1") == "1"


# ----------------------------------------------------------------------------
# Host-side preprocessing: sharding + edge bucketing
# ----------------------------------------------------------------------------
def _plan_layer(src_all, dloc_all, norm_all, core_all, src_chunk_of, src_idx_of,
                meta_np_dtype=np.float32):
    """Bucket each core's edges by (source-chunk k, dest-window w), pad each
    bucket to a multiple of 128 edges AND to identical chunk counts across
    all cores (SPMD requires one program).

    Returns (meta, per_core_arrays):
      meta = {"Tk": [T_0..T_3], "segs": [[(w, n_chunks), ...] per k]}
      per_core_arrays[c] = {"idx": [4 arrays int16 [16, T/16]],
                            "dst": [4 arrays f32 [128, T/128]],
                            "nrm": [4 arrays f32 [128, T/128]]}
    """
    counts = np.zeros((NC, NSRC, NW), dtype=np.int64)
    per_core = []
    for c in range(NC):
        sel = core_all == c
        src = src_all[sel]
        dloc = dloc_all[sel]
        nrm = norm_all[sel]
        k = src_chunk_of(src)
        w = dloc // WIN
        idxl = src_idx_of(src)
        order = np.lexsort((w, k))
        k, w, idxl, dloc, nrm = k[order], w[order], idxl[order], dloc[order], nrm[order]
        key = k * NW + w
        counts[c] = np.bincount(key, minlength=NSRC * NW).reshape(NSRC, NW)
        per_core.append((k, w, idxl, dloc, nrm, key))

    nch = (counts.max(axis=0) + 127) // 128          # [NSRC, NW] chunks per bucket
    Tk = (nch.sum(axis=1) * 128).astype(np.int64)    # padded edges per chunk k
    segs = [[(int(w), int(nch[k, w])) for w in range(NW) if nch[k, w] > 0]
            for k in range(NSRC)]

    # bucket base offsets inside each k-stream (in edges)
    base = np.zeros((NSRC, NW), dtype=np.int64)
    for k in range(NSRC):
        base[k] = np.concatenate(([0], np.cumsum(nch[k] * 128)[:-1]))

    out = []
    for c in range(NC):
        k, w, idxl, dloc, nrm, key = per_core[c]
        cnt = counts[c].reshape(-1)
        starts = np.concatenate(([0], np.cumsum(cnt)[:-1]))
        pos_in_bucket = np.arange(len(key)) - starts[key]
        tgt = base.reshape(-1)[key] + pos_in_bucket   # position inside k-stream
        arrs = {"idx": [], "dst": [], "nrm": []}
        for kk in range(NSRC):
            T = int(Tk[kk])
            idx16 = np.zeros(T, dtype=np.int16)
            dwf = np.full(T, -1.0, dtype=np.float32)
            nwf = np.zeros(T, dtype=np.float32)
            m = k == kk
            t = tgt[m]
            idx16[t] = idxl[m].astype(np.int16)
            dwf[t] = (dloc[m] - (w[m] * WIN)).astype(np.float32)
            nwf[t] = nrm[m]
            # device layouts: idx wraps by 16 (replicated to 128 partitions
            # for the 8 gpsimd cores), dst/nrm wrap by 128
            arrs["idx"].append(np.ascontiguousarray(
                np.tile(idx16.reshape(T // 16, 16).T, (8, 1))))
            arrs["dst"].append(np.ascontiguousarray(
                dwf.reshape(T // 128, 128).T.astype(meta_np_dtype)))
            arrs["nrm"].append(np.ascontiguousarray(
                nwf.reshape(T // 128, 128).T.astype(meta_np_dtype)))
        out.append(arrs)
    return {"Tk": [int(t) for t in Tk], "segs": segs}, out


def _preprocess(x, edge_index, W1, b1, W2, b2):
    row = np.asarray(edge_index[0], dtype=np.int64)
    col = np.asarray(edge_index[1], dtype=np.int64)
    deg = (np.bincount(col, minlength=N) + 1).astype(np.float32)  # + self loop
    dis = (1.0 / np.sqrt(deg)).astype(np.float32)

    loop = np.arange(N, dtype=np.int64)
    rows = np.concatenate([row, loop])
    cols = np.concatenate([col, loop])
    norms = (dis[rows] * dis[cols]).astype(np.float32)
    core = (cols // NS).astype(np.int64)
    dloc = cols - core * NS

    # L1 source chunks are INTERLEAVED (src % NSRC) rather than contiguous:
    # a core's self-loop sources all fall in one contiguous chunk (its own
    # shard), which would skew the cross-core bucket maxima and inflate the
    # SPMD padding by ~13%.
    meta1, arrs1 = _plan_layer(
        rows, dloc, norms, core,
        src_chunk_of=lambda s: s % NSRC,
        src_idx_of=lambda s: s // NSRC,
    )
    meta2, arrs2 = _plan_layer(
        rows, dloc, norms, core,
        src_chunk_of=lambda s: (s % NS) // Q4,
        src_idx_of=lambda s: (s // NS) * Q4 + (s % NS) % Q4,
        meta_np_dtype=np.float16,
    )

    x = np.ascontiguousarray(np.asarray(x, dtype=np.float32))
    shared = {
        "x": x,
        "W1": np.ascontiguousarray(np.asarray(W1, dtype=np.float32)),
        "b1": np.ascontiguousarray(np.asarray(b1, dtype=np.float32).reshape(1, D1)),
        "W2": np.ascontiguousarray(np.asarray(W2, dtype=np.float32)),
        "b2": np.ascontiguousarray(np.asarray(b2, dtype=np.float32).reshape(1, D2)),
    }
    in_maps = []
    for c in range(NC):
        m = dict(shared)
        for kk in range(NSRC):
            m[f"idx1_{kk}"] = arrs1[c]["idx"][kk]
            m[f"dst1_{kk}"] = arrs1[c]["dst"][kk]
            m[f"nrm1_{kk}"] = arrs1[c]["nrm"][kk]
            m[f"idx2_{kk}"] = arrs2[c]["idx"][kk]
            m[f"dst2_{kk}"] = arrs2[c]["dst"][kk]
            m[f"nrm2_{kk}"] = arrs2[c]["nrm"][kk]
        in_maps.append(m)
    return meta1, meta2, in_maps


# ----------------------------------------------------------------------------
# Device program
# ----------------------------------------------------------------------------
def _aggregate(nc, tc, mybir, meta, idx_d, dst_d, nrm_d, src_aps, acc, feat,
               dt_g, iota_t, tag):
    """Gather + P-matmul aggregation of one layer into SBUF acc [feat, NW*WIN].

    The P (one-hot * norm) selection tensor is built for a whole gather
    tile (GC chunks = G edges) with TWO vector ops, using stride-0
    broadcast access patterns on the per-edge dest/norm columns.
    """
    from contextlib import ExitStack

    import concourse.bass as bass

    f32 = mybir.dt.float32
    GC = G // 128
    with ExitStack() as ctx:
        mp = ctx.enter_context(tc.tile_pool(name=f"meta{tag}", bufs=2))
        gp = ctx.enter_context(tc.tile_pool(name=f"g{tag}", bufs=8))
        pp = ctx.enter_context(tc.tile_pool(name=f"p{tag}", bufs=4))
        psp = ctx.enter_context(tc.tile_pool(name=f"ps{tag}", bufs=4, space="PSUM"))

        def bcast(col_slice, mc):
            return bass.AP(col_slice.tensor, col_slice.offset,
                           [list(col_slice.ap[0]), [1, mc], [0, WIN]])

        ncalls = 0
        for k in range(NSRC):
            Tk = meta["Tk"][k]
            if Tk == 0:
                continue
            idx_t = mp.tile([128, Tk // 16], mybir.dt.int16, tag=f"idx{tag}")
            nc.sync.dma_start(idx_t[:], idx_d[k][:])
            dst_t = mp.tile([128, Tk // 128], dt_g, tag=f"dst{tag}")
            nc.sync.dma_start(dst_t[:], dst_d[k][:])
            nrm_t = mp.tile([128, Tk // 128], dt_g, tag=f"nrm{tag}")
            nc.sync.dma_start(nrm_t[:], nrm_d[k][:])
            jj = 0
            gt = None
            P8 = None
            for (w, nchk) in meta["segs"][k]:
                ps = psp.tile([feat, WIN], f32)
                for j in range(nchk):
                    g, slot = divmod(jj, GC)
                    if slot == 0:
                        mlen = min(G, Tk - g * G)
                        mc = mlen // 128
                        gt = gp.tile([128, GC, feat], dt_g, tag=f"gt{tag}")
                        nc.gpsimd.dma_gather(
                            gt[:, :mc, :],
                            src_aps[k],
                            idx_t[:, g * (G // 16): (g * G + mlen) // 16],
                            mlen, mlen, feat,
                            elem_step=src_aps[k].ap[0][0],
                            queue_num=ncalls % NQ,
                            single_packet=_SINGLE_PACKET,
                        )
                        ncalls += 1
                        P8 = pp.tile([128, GC, WIN], dt_g, tag=f"P{tag}")
                        nc.vector.tensor_tensor(
                            P8[:, :mc, :], iota_t[:, :mc, :],
                            bcast(dst_t[:, jj:jj + mc], mc),
                            mybir.AluOpType.is_equal)
                        nc.vector.tensor_tensor(
                            P8[:, :mc, :], P8[:, :mc, :],
                            bcast(nrm_t[:, jj:jj + mc], mc),
                            mybir.AluOpType.mult)
                    nc.tensor.matmul(ps[:], gt[:, slot, :], P8[:, slot, :],
                                     start=(j == 0), stop=(j == nchk - 1))
                    jj += 1
                nc.vector.tensor_tensor(
                    acc[:, w * WIN:(w + 1) * WIN],
                    acc[:, w * WIN:(w + 1) * WIN], ps[:],
                    mybir.AluOpType.add)


def _build(meta1, meta2, debug=False, stage="full"):
    from contextlib import ExitStack

    import concourse.bacc as bacc
    import concourse.mybir as mybir
    import concourse.tile as tile

    f32, f16, i16 = mybir.dt.float32, mybir.dt.float16, mybir.dt.int16

    nc = bacc.Bacc("TRN2", target_bir_lowering=False, debug=debug,
                   num_devices=NC, num_swdge_queues=NQ)

    x_d = nc.dram_tensor("x", [N, D0], f32, kind="ExternalInput")
    w1_d = nc.dram_tensor("W1", [D0, D1], f32, kind="ExternalInput")
    b1_d = nc.dram_tensor("b1", [1, D1], f32, kind="ExternalInput")
    w2_d = nc.dram_tensor("W2", [D1, D2], f32, kind="ExternalInput")
    b2_d = nc.dram_tensor("b2", [1, D2], f32, kind="ExternalInput")

    idx1_d, dst1_d, nrm1_d, idx2_d, dst2_d, nrm2_d = [], [], [], [], [], []
    for k in range(NSRC):
        T1, T2 = meta1["Tk"][k], meta2["Tk"][k]
        idx1_d.append(nc.dram_tensor(f"idx1_{k}", [128, T1 // 16], i16, kind="ExternalInput"))
        dst1_d.append(nc.dram_tensor(f"dst1_{k}", [128, T1 // 128], f32, kind="ExternalInput"))
        nrm1_d.append(nc.dram_tensor(f"nrm1_{k}", [128, T1 // 128], f32, kind="ExternalInput"))
        idx2_d.append(nc.dram_tensor(f"idx2_{k}", [128, T2 // 16], i16, kind="ExternalInput"))
        dst2_d.append(nc.dram_tensor(f"dst2_{k}", [128, T2 // 128], f16, kind="ExternalInput"))
        nrm2_d.append(nc.dram_tensor(f"nrm2_{k}", [128, T2 // 128], f16, kind="ExternalInput"))

    h_own = nc.dram_tensor("h_own", [NS, D1], f16, kind="Internal")
    hf = [nc.dram_tensor(f"hf{q}", [NC * Q4, D1], f16, kind="Internal",
                         addr_space="Shared") for q in range(NSRC)]
    if stage == "A":
        out_d = nc.dram_tensor("out", [D0, NW * WIN], f32, kind="ExternalOutput")
    elif stage == "AB":
        out_d = nc.dram_tensor("out", [NS, D1], f16, kind="ExternalOutput")
    elif stage == "ABC":
        out_d = nc.dram_tensor("out", [NC * Q4, D1], f16, kind="ExternalOutput")
    elif stage == "ABCD":
        out_d = nc.dram_tensor("out", [D1, NW * WIN], f32, kind="ExternalOutput")
    else:
        out_d = nc.dram_tensor("out", [NS, D2], f32, kind="ExternalOutput")

    with tile.TileContext(nc) as tc:
        with ExitStack() as top:
            const = top.enter_context(tc.tile_pool(name="const", bufs=1))
            w1_t = const.tile([D0, D1], f32)
            nc.sync.dma_start(w1_t[:], w1_d[:])
            b1_t = const.tile([1, D1], f32)
            nc.sync.dma_start(b1_t[:], b1_d[:])
            w2_t = const.tile([D1, D2], f32)
            nc.sync.dma_start(w2_t[:], w2_d[:])
            b2_t = const.tile([1, D2], f32)
            nc.sync.dma_start(b2_t[:], b2_d[:])
            GC = G // 128
            iota32 = const.tile([128, GC, WIN], f32)
            nc.gpsimd.iota(iota32[:], pattern=[[0, GC], [1, WIN]], base=0,
                           channel_multiplier=0,
                           allow_small_or_imprecise_dtypes=True)
            iota16 = const.tile([128, GC, WIN], f16)
            nc.gpsimd.iota(iota16[:], pattern=[[0, GC], [1, WIN]], base=0,
                           channel_multiplier=0,
                           allow_small_or_imprecise_dtypes=True)
            ones_t = const.tile([1, 128], f32)
            nc.vector.memset(ones_t[:], 1.0)

            accp = top.enter_context(tc.tile_pool(name="acc", bufs=1))
            acc2 = accp.tile([D1, NW * WIN], f32)

            # ---- Layer 1: aggregate into acc1 [D0, NW*WIN] ----
            with tc.tile_pool(name="acc1", bufs=1) as acc1p:
                acc1 = acc1p.tile([D0, NW * WIN], f32)
                nc.vector.memset(acc1[:], 0.0)
                import concourse.bass as bass
                xb = x_d[:]
                x_srcs = [bass.AP(xb.tensor, k * D0,
                                  [[NSRC * D0, SC1], [1, D0]])
                          for k in range(NSRC)]
                _aggregate(nc, tc, mybir, meta1, idx1_d, dst1_d, nrm1_d,
                           x_srcs, acc1, D0, f32, iota32, "A")

                if stage == "A":
                    nc.sync.dma_start(out_d[:], acc1[:])
                else:
                    # ---- h = relu(acc1.T @ W1 + b1), store fp16 ----
                    with ExitStack() as sb:
                        hp = sb.enter_context(tc.tile_pool(name="hb", bufs=4))
                        psb = sb.enter_context(tc.tile_pool(name="psb", bufs=4, space="PSUM"))
                        for w in range(NW):
                            M = min(WIN, NS - w * WIN)
                            ps = psb.tile([M, D1], f32, tag="psb")
                            nc.tensor.matmul(ps[:], acc1[:, w * WIN:w * WIN + M],
                                             w1_t[:], start=True, stop=False)
                            nc.tensor.matmul(ps[:], ones_t[:, :M], b1_t[:],
                                             start=False, stop=True)
                            ht = hp.tile([M, D1], f16, tag="ht")
                            nc.scalar.activation(ht[:], ps[:],
                                                 mybir.ActivationFunctionType.Relu)
                            nc.sync.dma_start(h_own[w * WIN:w * WIN + M, :], ht[:])

            if stage == "AB":
                nc.sync.dma_start(out_d[:], h_own[:])
            elif stage != "A":
                nc.vector.memset(acc2[:], 0.0)

                # ---- AllGather h (4 node-range chunks) ----
                for q in range(NSRC):
                    nc.gpsimd.collective_compute(
                        "AllGather", mybir.AluOpType.bypass,
                        replica_groups=[list(range(NC))],
                        ins=[h_own[q * Q4:(q + 1) * Q4, :]],
                        outs=[hf[q][:, :]],
                    )

                if stage == "ABC":
                    nc.sync.dma_start(out_d[:], hf[0][:])
                else:
                    # ---- Layer 2: aggregate into acc2 [D1, NW*WIN] ----
                    _aggregate(nc, tc, mybir, meta2, idx2_d, dst2_d, nrm2_d,
                               [hf[k][:, :] for k in range(NSRC)],
                               acc2, D1, f16, iota16, "B")

                    if stage == "ABCD":
                        nc.sync.dma_start(out_d[:], acc2[:])
                    else:
                        # ---- out = acc2.T @ W2 + b2 ----
                        with ExitStack() as sb:
                            op = sb.enter_context(tc.tile_pool(name="ob", bufs=4))
                            pso = sb.enter_context(tc.tile_pool(name="pso", bufs=4, space="PSUM"))
                            for w in range(NW):
                                M = min(WIN, NS - w * WIN)
                                ps = pso.tile([M, D2], f32, tag="pso")
                                nc.tensor.matmul(ps[:], acc2[:, w * WIN:w * WIN + M],
                                                 w2_t[:], start=True, stop=False)
                                nc.tensor.matmul(ps[:], ones_t[:, :M], b2_t[:],
                                                 start=False, stop=True)
                                ot = op.tile([M, D2], f32, tag="ot")
                                nc.vector.tensor_copy(ot[:], ps[:])
                                nc.sync.dma_start(out_d[w * WIN:w * WIN + M, :], ot[:])

    nc.compile()
    return nc


# ----------------------------------------------------------------------------
# Entry point
# ----------------------------------------------------------------------------
def _ensure_axon_hooks_module():
    """bass_utils hard-imports antenv.axon_hooks when BASS_TRACE is set;
    provide a degradable stub if the image's antenv lacks it."""
    import types

    try:
        import antenv.axon_hooks  # noqa: F401
        return
    except ImportError:
        pass
    try:
        import antenv
    except ImportError:
        return
    mod = types.ModuleType("antenv.axon_hooks")
    mod._hook = None
    mod.set_axon_ntff_profile_hook = lambda h: setattr(mod, "_hook", h)
    mod.get_axon_ntff_profile_hook = lambda: mod._hook
    sys.modules["antenv.axon_hooks"] = mod
    antenv.axon_hooks = mod


def kernel(x, edge_index, W1, b1, W2, b2):
    _ensure_axon_hooks_module()
    from concourse import bass_utils

    meta1, meta2, in_maps = _preprocess(x, edge_index, W1, b1, W2, b2)
    nc = _build(meta1, meta2, debug=False)
    res = bass_utils.run_bass_kernel_spmd(nc, in_maps, core_ids=list(range(NC)))
    out = np.concatenate([r["out"] for r in res.results], axis=0)
    return out.astype(np.float32)
